# revision 27
# baseline (speedup 1.0000x reference)
"""Distributed Bass kernel for nn_AttentionLayer (2-branch GAT-style layer).

Row-shard over 8 NeuronCores (512 rows each), transposed on-chip layout
(k on partitions, own-row i on free axis) so masked softmax feeds the PE
attention matmuls without transposes.

v3 design:
- Full adj AND full h^T are REPLICATED to every core in HBM.  No mid-
  kernel collectives at all (the first collective's implicit device
  barrier costs ~76us and serializes the CC stream); a dummy 16-byte
  AllGather is fired at t=0 so the barrier overlaps the whole kernel and
  the final BN-stats AllReduce launches instantly.
- Wh^T is computed per 512-column chunk with float32r matmuls (1 cyc/row)
  and cast to bf16 rows [Wh^T | ones]; per-k-tile stationaries
  [128, 80] for the attention matmul come from XBAR DMA transposes
  (zero PE cost).  The ones row makes each attention matmul also emit
  the softmax denominator (psum row 64).
- s1/s2 come from V = W @ a folded on the host: s = h @ V, computed in
  f32r on PE (exact enough), so softmax starts ~15us into the kernel.
- lrelu fused via Prelu (alpha=0.2) which shares the activation table
  set with Exp/Square (no table reloads); masking via the +-40 bias
  trick with -40 as the Exp activation bias.
- adj2 counts (2-hop) on PE in fp8 DoubleRow, exact in f32 psum;
  attention matmuls run one group behind (software pipelining).
"""

import sys
import numpy as np

for _p in ("/opt/trn_rl_repo", "/opt/trn_rl_repo/concourse"):
    if _p not in sys.path:
        sys.path.insert(0, _p)

import ml_dtypes

N = 4096
M_CORES = 8
R = N // M_CORES          # 512 rows per core
IN_F = 512
HALF = IN_F // 2          # 256
F = 64
P = 128                   # partitions
NT = N // P               # 32 k tiles
NTP = NT // 2             # 16 DoubleRow k-tile pairs
G = 4                     # k-tiles per psum group
NG = NT // G              # 8 groups
WROWS = 80                # whTf rows incl ones row + pad (mult of 16)
ALPHA = 0.2
EPS = 1e-5
BIG2 = 40.0               # mask bias; exp(e - 40) ~ 0 for e <= ~12
INV_N = 1.0 / N

_CACHED = {}


def build_nc():
    from concourse import bacc, tile, mybir

    f32 = mybir.dt.float32
    f32r = mybir.dt.float32r
    bf16 = mybir.dt.bfloat16
    fp8 = mybir.dt.float8e4
    Alu = mybir.AluOpType
    Act = mybir.ActivationFunctionType
    DR = mybir.MatmulPerfMode.DoubleRow

    nc = bacc.Bacc("TRN2", target_bir_lowering=False, debug=False,
                   num_devices=M_CORES)

    hT_p = nc.declare_dram_parameter("hT", [IN_F, R], f32r, isOutput=False)
    hTf_p = nc.declare_dram_parameter("hTf", [IN_F, N], f32r, isOutput=False)
    adjdr_p = nc.declare_dram_parameter("adjdr", [NTP * P, 2, N], fp8,
                                        isOutput=False)
    adjT_p = nc.declare_dram_parameter("adjT", [NTP * P, 2, R], fp8,
                                       isOutput=False)
    dts_p = nc.declare_dram_parameter("dts", [N, R], bf16, isOutput=False)
    W_p = nc.declare_dram_parameter("W12", [HALF, 2 * F], f32r, isOutput=False)
    V4_p = nc.declare_dram_parameter("V4", [IN_F, 4], f32r, isOutput=False)
    gb_p = nc.declare_dram_parameter("gb", [F, 4], f32, isOutput=False)
    id_p = nc.declare_dram_parameter("ident", [P, P], f32, isOutput=False)
    onesrow_p = nc.declare_dram_parameter("onesrow", [WROWS - F, N], bf16,
                                          isOutput=False)
    ones1_p = nc.declare_dram_parameter("ones1r", [1, P], f32r,
                                        isOutput=False)
    out_p = nc.declare_dram_parameter("out", [R, 2 * F], f32, isOutput=True)

    RG = [list(range(M_CORES))]
    RQ = R // P               # 4 row blocks per core

    with tile.TileContext(nc) as tc:
        with (
            tc.tile_pool(name="sb", bufs=1) as sb,
            tc.tile_pool(name="aft", bufs=1) as aft,
            tc.tile_pool(name="sbt", bufs=3) as sbt,
            tc.tile_pool(name="psA", bufs=1, space="PSUM") as psA,
            tc.tile_pool(name="psC", bufs=6, space="PSUM") as psC,
            tc.tile_pool(name="dram", bufs=1, space="DRAM") as dram,
        ):
            # ---- dummy collective first: absorbs the one-time device
            # barrier + CC stream setup while local compute proceeds.
            # (collectives cannot read IO tensors -> bounce via sbuf)
            warm_sb = sb.tile([1, 4], f32)
            nc.vector.memset(warm_sb[:], 0.0)
            warm_in = dram.tile([1, 4], f32, name="warm_in")
            nc.gpsimd.dma_start(warm_in[:], warm_sb[:])
            dummy_out = dram.tile([M_CORES, 4], f32, addr_space="Shared",
                                  name="dummy_out")
            nc.gpsimd.collective_compute(
                "AllGather", Alu.bypass, replica_groups=RG,
                ins=[warm_in[:].opt()], outs=[dummy_out[:].opt()])

            # ---- small persistent loads (sync queue) ----
            ident = sb.tile([P, P], f32)
            nc.sync.dma_start(ident[:], id_p[:])
            V4_sb = []
            for t in range(4):
                v = sb.tile([P, 4], f32r, tag=f"v4_{t}")
                nc.sync.dma_start(v[:], V4_p[P * t:P * (t + 1), :])
                V4_sb.append(v)
            gb_sb = sb.tile([F, 4], f32)
            nc.sync.dma_start(gb_sb[:], gb_p[:])
            W_sb = []
            for t in range(2):
                w = sb.tile([P, 2 * F], f32r, tag=f"w{t}")
                nc.sync.dma_start(w[:], W_p[P * t:P * (t + 1), :])
                W_sb.append(w)
            hT_sb = []
            for t in range(4):
                ht = sb.tile([P, R], f32r, tag=f"ht{t}")
                nc.sync.dma_start(ht[:], hT_p[P * t:P * (t + 1), :])
                hT_sb.append(ht)
            ones1 = sb.tile([1, P], f32r)
            nc.sync.dma_start(ones1[:], ones1_p[:])
            neg40 = sb.tile([P, 1], f32)
            nc.vector.memset(neg40[:], -BIG2)
            ones64 = sb.tile([F + 1, F], f32)
            nc.vector.memset(ones64[F:F + 1, :], 1.0)

            # ---- adjT shard: evens on sync, odds on gpsimd ----
            adjT_sb = []
            for t in range(NTP):
                at = sb.tile([P, 2, R], fp8, tag=f"adjT{t}")
                q = nc.sync if t % 2 == 0 else nc.gpsimd
                q.dma_start(at[:], adjT_p[P * t:P * (t + 1), :, :])
                adjT_sb.append(at)

            # ---- whTf (bf16, transposed Wh + ones row + pad) ----
            whTf_bf = []
            for b in range(2):
                wt = sb.tile([WROWS, N], bf16, tag=f"whTf{b}")
                nc.sync.dma_start(wt[F:WROWS, :], onesrow_p[:])
                whTf_bf.append(wt)

            # ---- af tiles (sync: even T, gpsimd: odd T) ----
            af_tiles = {}

            def load_af(g):
                for t in range(NTP):
                    af = aft.tile([P, 2, R], fp8, tag="af", bufs=48,
                                  name=f"af{g}_{t}")
                    q = nc.sync if t % 2 == 0 else nc.gpsimd
                    q.dma_start(af[:],
                                adjdr_p[P * t:P * (t + 1), :,
                                        R * g:R * (g + 1)])
                    af_tiles[(g, t)] = af

            load_af(0)

            # ---- own-rows preamble: s1 and its broadcast ----
            s1bc = []
            for b in range(2):
                svo = psC.tile([1, R], f32, tag="cnt", name=f"svo{b}")
                for t in range(4):
                    nc.tensor.matmul(svo[:],
                                     V4_sb[t][:, b:b + 1],
                                     hT_sb[t][:],
                                     start=(t == 0), stop=(t == 3))
                sc = sb.tile([1, R], f32r, tag=f"sc{b}")
                nc.vector.tensor_copy(sc[:], svo[:])
                bc = psC.tile([P, R], f32, tag="cnt", name=f"s1bc_ps{b}")
                nc.tensor.matmul(bc[:], ones1[:],
                                 sc[:], start=True, stop=True)
                s1b = sb.tile([P, R], f32, tag=f"s1bc{b}")
                nc.vector.tensor_copy(s1b[:], bc[:])
                s1bc.append(s1b)

            s2_sb = []
            for b in range(2):
                s2t = sb.tile([P, NT], f32, tag=f"s2_{b}", name=f"s2sb{b}")
                s2_sb.append(s2t)
            whf_t = [[None] * NT, [None] * NT]

            # ---- attention accumulators (psum rows 0:64 out, 64 sums) ----
            accT = []
            for b in range(2):
                acc_t = psA.tile([F + 1, R], f32, tag=f"acc{b}",
                                 name=f"accT{b}")
                accT.append(acc_t)

            pt_b1 = {}
            pt_b2 = {}

            def chunk_wh(g):
                """Wh^T and s-vectors for k-window g; whf stationaries."""
                hf = []
                for t in range(4):
                    h = sbt.tile([P, R], f32r, tag="hf", bufs=8)
                    nc.scalar.dma_start(
                        h[:], hTf_p[P * t:P * (t + 1), R * g:R * (g + 1)])
                    hf.append(h)
                sv_ps = psC.tile([4, R], f32, tag="cnt", name=f"sv{g}")
                for t in range(4):
                    nc.tensor.matmul(sv_ps[:], V4_sb[t][:],
                                     hf[t][:],
                                     start=(t == 0), stop=(t == 3))
                wh_ps = []
                for b in range(2):
                    whb = psC.tile([F, R], f32, tag="cnt", name=f"wh{g}_{b}")
                    for t in range(2):
                        nc.tensor.matmul(
                            whb[:],
                            W_sb[t][:, F * b:F * (b + 1)],
                            hf[2 * b + t][:],
                            start=(t == 0), stop=(t == 1))
                    wh_ps.append(whb)
                sv_sb = sbt.tile([4, R], f32, tag="svsb", bufs=2)
                nc.vector.tensor_copy(sv_sb[:], sv_ps[:])
                s2d = dram.tile([2, R], f32, name=f"s2d{g}")
                nc.sync.dma_start(s2d[:], sv_sb[2:4, :])
                for b in range(2):
                    nc.vector.tensor_copy(
                        whTf_bf[b][0:F, R * g:R * (g + 1)],
                        wh_ps[b][:])
                    nc.sync.dma_start(
                        s2_sb[b][:, G * g:G * (g + 1)],
                        s2d[b].rearrange("(q p) -> p q", p=P))
                    for j in range(G):
                        kt = G * g + j
                        wf = sb.tile([P, WROWS], bf16, tag=f"whf{b}_{kt}")
                        nc.sync.dma_start(
                            wf[:], whTf_bf[b][:, P * kt:P * (kt + 1)],
                            transpose=True)
                        whf_t[b][kt] = wf

            def softmax_b2(g, j, cnt):
                kt = G * g + j
                dt_t = sbt.tile([P, R], bf16, tag="dt", bufs=6)
                nc.sync.dma_start(dt_t[:], dts_p[P * kt:P * (kt + 1), :])
                e2 = sbt.tile([P, R], f32, tag="e", bufs=4)
                nc.scalar.activation(e2[:], s1bc[1][:], Act.Prelu,
                                     bias=s2_sb[1][:, kt:kt + 1],
                                     alpha=ALPHA)
                m = sbt.tile([P, R], f32, tag="m", bufs=8)
                nc.vector.tensor_scalar(m[:], cnt[:], 1.0, BIG2,
                                        op0=Alu.min, op1=Alu.mult)
                nc.vector.tensor_tensor(m[:], m[:], dt_t[:], op=Alu.add)
                nc.vector.tensor_tensor(m[:], m[:], e2[:], op=Alu.add)
                pt = sbt.tile([P, R], bf16, tag="pt", bufs=18)
                nc.scalar.activation(pt[:], m[:], Act.Exp, bias=neg40[:])
                pt_b2[kt] = pt

            def softmax_b1(g, j):
                kt = G * g + j
                e1 = sbt.tile([P, R], f32, tag="e", bufs=4)
                nc.scalar.activation(e1[:], s1bc[0][:], Act.Prelu,
                                     bias=s2_sb[0][:, kt:kt + 1],
                                     alpha=ALPHA)
                z = sbt.tile([P, R], f32, tag="m", bufs=8)
                nc.vector.scalar_tensor_tensor(
                    z[:], adjT_sb[kt // 2][:, kt % 2, :], BIG2, e1[:],
                    op0=Alu.mult, op1=Alu.add)
                pt = sbt.tile([P, R], bf16, tag="pt", bufs=18)
                nc.scalar.activation(pt[:], z[:], Act.Exp, bias=neg40[:])
                pt_b1[kt] = pt

            def emit_att(g):
                for j in range(G):
                    kt = G * g + j
                    nc.tensor.matmul(accT[0][:],
                                     whf_t[0][kt][:, 0:F + 1], pt_b1[kt][:],
                                     start=(kt == 0), stop=(kt == NT - 1))
                    nc.tensor.matmul(accT[1][:],
                                     whf_t[1][kt][:, 0:F + 1], pt_b2[kt][:],
                                     start=(kt == 0), stop=(kt == NT - 1))

            # ---- main loop ----
            for g in range(NG):
                if g + 1 < NG:
                    load_af(g + 1)
                chunk_wh(g)
                cnts = [psC.tile([P, R], f32, tag="cnt", name=f"cnt{g}_{j}")
                        for j in range(G)]
                for t in range(NTP):
                    af = af_tiles.pop((g, t))
                    for j in range(G):
                        nc.tensor.matmul(cnts[j][:],
                                         af[:, :, P * j:P * (j + 1)],
                                         adjT_sb[t][:],
                                         perf_mode=DR,
                                         start=(t == 0), stop=(t == NTP - 1))
                for j in range(G):
                    softmax_b2(g, j, cnts[j])
                for j in range(G):
                    softmax_b1(g, j)
                if g >= 1:
                    emit_att(g - 1)
            emit_att(NG - 1)

            # ---- epilogue: normalize, BN stats + AllReduce, BN+lrelu ----
            hp = []
            for b in range(2):
                srec = sb.tile([F + 1, R], f32, tag=f"srec{b}")
                nc.vector.tensor_copy(srec[F:F + 1, :], accT[b][F:F + 1, :])
                rrec = sb.tile([F + 1, R], f32, tag=f"rrec{b}")
                nc.vector.reciprocal(rrec[F:F + 1, :], srec[F:F + 1, :])
                bc_ps = psC.tile([F, R], f32, tag="cnt", name=f"bc_ps{b}")
                nc.tensor.matmul(bc_ps[:], ones64[F:F + 1, :],
                                 rrec[F:F + 1, :],
                                 start=True, stop=True)
                bc_sb = sb.tile([F, R], f32, tag=f"bcs{b}")
                nc.vector.tensor_copy(bc_sb[:], bc_ps[:])
                hp_b = sb.tile([F, R], f32, tag=f"hp{b}")
                nc.vector.tensor_tensor(hp_b[:], accT[b][0:F, :], bc_sb[:],
                                        op=Alu.mult)
                hp.append(hp_b)

            # stats packed [64, 4]: (sum1, sumsq1, sum2, sumsq2)
            sx = sb.tile([F, 4], f32)
            sq = sb.tile([F, R], bf16)
            for b in range(2):
                nc.vector.tensor_reduce(sx[:, 2 * b:2 * b + 1], hp[b][:],
                                        axis=mybir.AxisListType.X,
                                        op=Alu.add)
                nc.scalar.activation(sq[:], hp[b][:], Act.Square,
                                     accum_out=sx[:, 2 * b + 1:2 * b + 2])
            stats_in = dram.tile([F, 4], f32, name="stats_in")
            nc.sync.dma_start(stats_in[:], sx[:])
            stats_out = dram.tile([F, 4], f32, addr_space="Shared",
                                  name="stats_out")
            nc.gpsimd.collective_compute(
                "AllReduce", Alu.add, replica_groups=RG,
                ins=[stats_in[:].opt()], outs=[stats_out[:].opt()])
            gst = sb.tile([F, 4], f32)
            nc.sync.dma_start(gst[:], stats_out[:])

            gst3 = gst[:].rearrange("f (b s) -> f b s", b=2)
            mean = sb.tile([F, 2], f32)
            nc.scalar.mul(mean[:], gst3[:, :, 0], INV_N)
            ex2 = sb.tile([F, 2], f32)
            nc.scalar.mul(ex2[:], gst3[:, :, 1], INV_N)
            var = sb.tile([F, 2], f32)
            nc.vector.scalar_tensor_tensor(var[:], mean[:], -1.0, mean[:],
                                           op0=Alu.mult, op1=Alu.mult)
            nc.vector.tensor_add(var[:], var[:], ex2[:])
            nc.vector.tensor_scalar_add(var[:], var[:], EPS)
            std = sb.tile([F, 2], f32)
            nc.scalar.activation(std[:], var[:], Act.Sqrt)
            rstd = sb.tile([F, 2], f32)
            nc.vector.reciprocal(rstd[:], std[:])
            gb3 = gb_sb[:].rearrange("f (b s) -> f b s", b=2)
            scale = sb.tile([F, 2], f32)
            nc.vector.tensor_mul(scale[:], gb3[:, :, 0], rstd[:])
            nbias = sb.tile([F, 2], f32)
            nc.vector.scalar_tensor_tensor(nbias[:], mean[:], -1.0, scale[:],
                                           op0=Alu.mult, op1=Alu.mult)
            nc.vector.tensor_add(nbias[:], nbias[:], gb3[:, :, 1])

            # fused BN apply + lrelu; transpose out per branch
            ob = sb.tile([P, RQ, 2 * F], f32)
            for b in range(2):
                finb = sb.tile([F, R], f32, tag=f"fin{b}")
                nc.scalar.activation(finb[:], hp[b][:], Act.Prelu,
                                     bias=nbias[:, b:b + 1],
                                     scale=scale[:, b:b + 1], alpha=ALPHA)
                for q in range(RQ):
                    tp = psC.tile([P, F], f32, tag="cnt", name=f"otp{b}_{q}")
                    nc.tensor.transpose(tp[:], finb[:, P * q:P * (q + 1)],
                                        ident[0:F, 0:F])
                    nc.vector.tensor_copy(ob[:, q, F * b:F * (b + 1)],
                                          tp[:])
            nc.sync.dma_start(
                out_p.rearrange("(q p) f -> p q f", p=P), ob[:])

    nc.compile()
    return nc


def _get_nc():
    if "nc" not in _CACHED:
        _CACHED["nc"] = build_nc()
    return _CACHED["nc"]


def make_in_maps(h, adj, W1, W2, a, gamma, beta):
    h = np.asarray(h, dtype=np.float32)
    adj = np.asarray(adj, dtype=np.float32)
    W1 = np.asarray(W1, np.float32)
    W2 = np.asarray(W2, np.float32)
    W12 = np.concatenate([W1, W2], axis=1)
    a_flat = np.asarray(a, np.float32).reshape(2 * F)
    a1, a2 = a_flat[:F], a_flat[F:]
    # V4 = folded W @ a vectors: s = h @ V4 gives s1/s2 for both branches
    V4 = np.zeros((IN_F, 4), dtype=np.float32)
    V4[:HALF, 0] = W1 @ a1
    V4[HALF:, 1] = W2 @ a1
    V4[:HALF, 2] = W1 @ a2
    V4[HALF:, 3] = W2 @ a2
    gamma = np.asarray(gamma, np.float32)
    beta = np.asarray(beta, np.float32)
    gb = np.stack([gamma[:F], beta[:F], gamma[F:], beta[F:]], axis=1)
    ident = np.eye(P, dtype=np.float32)
    onesrow = np.zeros((WROWS - F, N), dtype=ml_dtypes.bfloat16)
    onesrow[0, :] = 1.0

    fp8 = ml_dtypes.float8_e4m3fn
    adj_f8 = adj.astype(fp8)
    # full adj in DoubleRow layout: adjdr[128T+p, s, k] = adj[256T+128s+p, k]
    adjdr = np.ascontiguousarray(
        adj_f8.reshape(NTP, 2, P, N).transpose(0, 2, 1, 3)
        .reshape(NTP * P, 2, N))
    hTf = np.ascontiguousarray(h.T)

    in_maps = []
    for c in range(M_CORES):
        r0 = c * R
        shT = np.ascontiguousarray(adj[r0:r0 + R, :].T).astype(fp8)
        adjT = np.ascontiguousarray(
            shT.reshape(NTP, 2, P, R).transpose(0, 2, 1, 3)
            .reshape(NTP * P, 2, R))
        dts = np.zeros((N, R), dtype=ml_dtypes.bfloat16)
        dts[np.arange(r0, r0 + R), np.arange(R)] = -BIG2
        in_maps.append({
            "hT": np.ascontiguousarray(h[r0:r0 + R, :].T),
            "hTf": hTf,
            "adjdr": adjdr,
            "adjT": adjT,
            "dts": dts,
            "W12": W12,
            "V4": V4,
            "gb": gb,
            "ident": ident,
            "onesrow": onesrow,
            "ones1r": np.ones((1, P), dtype=np.float32),
        })
    return in_maps


def kernel(h, adj, W1, W2, a, gamma, beta):
    from concourse.bass_utils import run_bass_kernel_spmd

    in_maps = make_in_maps(h, adj, W1, W2, a, gamma, beta)
    nc = _get_nc()
    res = run_bass_kernel_spmd(nc, in_maps, core_ids=list(range(M_CORES)))
    outs = [np.asarray(res.results[c]["out"]) for c in range(M_CORES)]
    return np.concatenate(outs, axis=0)


# revision 29
# speedup vs baseline: 1.0412x; 1.0412x over previous
"""Distributed Bass kernel for nn_AttentionLayer (2-branch GAT-style layer).

Row-shard over 8 NeuronCores (512 rows each), transposed on-chip layout
(k on partitions, own-row i on free axis) so masked softmax feeds the PE
attention matmuls without transposes.

v3 design:
- Full adj AND full h^T are REPLICATED to every core in HBM.  No mid-
  kernel collectives at all (the first collective's implicit device
  barrier costs ~76us and serializes the CC stream); a dummy 16-byte
  AllGather is fired at t=0 so the barrier overlaps the whole kernel and
  the final BN-stats AllReduce launches instantly.
- Wh^T is computed per 512-column chunk with float32r matmuls (1 cyc/row)
  and cast to bf16 rows [Wh^T | ones]; per-k-tile stationaries
  [128, 80] for the attention matmul come from XBAR DMA transposes
  (zero PE cost).  The ones row makes each attention matmul also emit
  the softmax denominator (psum row 64).
- s1/s2 come from V = W @ a folded on the host: s = h @ V, computed in
  f32r on PE (exact enough), so softmax starts ~15us into the kernel.
- lrelu fused via Prelu (alpha=0.2) which shares the activation table
  set with Exp/Square (no table reloads); masking via the +-40 bias
  trick with -40 as the Exp activation bias.
- adj2 counts (2-hop) on PE in fp8 DoubleRow, exact in f32 psum;
  attention matmuls run one group behind (software pipelining).
"""

import sys
import numpy as np

for _p in ("/opt/trn_rl_repo", "/opt/trn_rl_repo/concourse"):
    if _p not in sys.path:
        sys.path.insert(0, _p)

import ml_dtypes

N = 4096
M_CORES = 8
R = N // M_CORES          # 512 rows per core
IN_F = 512
HALF = IN_F // 2          # 256
F = 64
P = 128                   # partitions
NT = N // P               # 32 k tiles
NTP = NT // 2             # 16 DoubleRow k-tile pairs
G = 4                     # k-tiles per psum group
NG = NT // G              # 8 groups
WROWS = 80                # whTf rows incl ones row + pad (mult of 16)
ALPHA = 0.2
EPS = 1e-5
BIG2 = 40.0               # mask bias; exp(e - 40) ~ 0 for e <= ~12
INV_N = 1.0 / N

_CACHED = {}


def build_nc():
    from concourse import bacc, tile, mybir

    f32 = mybir.dt.float32
    f32r = mybir.dt.float32r
    bf16 = mybir.dt.bfloat16
    fp8 = mybir.dt.float8e4
    Alu = mybir.AluOpType
    Act = mybir.ActivationFunctionType
    DR = mybir.MatmulPerfMode.DoubleRow

    nc = bacc.Bacc("TRN2", target_bir_lowering=False, debug=False,
                   num_devices=M_CORES)

    hT_p = nc.declare_dram_parameter("hT", [IN_F, R], f32r, isOutput=False)
    hTf_p = nc.declare_dram_parameter("hTf", [IN_F, N], f32r, isOutput=False)
    adjdr_p = nc.declare_dram_parameter("adjdr", [NTP * P, 2, N], fp8,
                                        isOutput=False)
    adjT_p = nc.declare_dram_parameter("adjT", [NTP * P, 2, R], fp8,
                                       isOutput=False)
    dts_p = nc.declare_dram_parameter("dts", [N, R], bf16, isOutput=False)
    W_p = nc.declare_dram_parameter("W12", [HALF, 2 * F], f32r, isOutput=False)
    V4_p = nc.declare_dram_parameter("V4", [IN_F, 4], f32r, isOutput=False)
    gb_p = nc.declare_dram_parameter("gb", [F, 4], f32, isOutput=False)
    id_p = nc.declare_dram_parameter("ident", [P, P], f32, isOutput=False)
    ones1_p = nc.declare_dram_parameter("ones1r", [1, P], f32r,
                                        isOutput=False)
    out_p = nc.declare_dram_parameter("out", [R, 2 * F], f32, isOutput=True)

    RG = [list(range(M_CORES))]
    RQ = R // P               # 4 row blocks per core

    with tile.TileContext(nc) as tc:
        with (
            tc.tile_pool(name="sb", bufs=1) as sb,
            tc.tile_pool(name="aft", bufs=1) as aft,
            tc.tile_pool(name="sbt", bufs=3) as sbt,
            tc.tile_pool(name="psA", bufs=1, space="PSUM") as psA,
            tc.tile_pool(name="psC", bufs=6, space="PSUM") as psC,
            tc.tile_pool(name="dram", bufs=1, space="DRAM") as dram,
        ):
            # ---- dummy collective first: absorbs the one-time device
            # barrier + CC stream setup while local compute proceeds.
            # (collectives cannot read IO tensors -> bounce via sbuf)
            warm_sb = sb.tile([1, 4], f32)
            nc.vector.memset(warm_sb[:], 0.0)
            warm_in = dram.tile([1, 4], f32, name="warm_in")
            nc.gpsimd.dma_start(warm_in[:], warm_sb[:])
            dummy_out = dram.tile([M_CORES, 4], f32, addr_space="Shared",
                                  name="dummy_out")
            nc.gpsimd.collective_compute(
                "AllGather", Alu.bypass, replica_groups=RG,
                ins=[warm_in[:].opt()], outs=[dummy_out[:].opt()])

            # ---- small persistent loads (sync queue) ----
            ident = sb.tile([P, P], f32)
            nc.sync.dma_start(ident[:], id_p[:])
            V4_sb = []
            for t in range(4):
                v = sb.tile([P, 4], f32r, tag=f"v4_{t}")
                nc.sync.dma_start(v[:], V4_p[P * t:P * (t + 1), :])
                V4_sb.append(v)
            gb_sb = sb.tile([F, 4], f32)
            nc.sync.dma_start(gb_sb[:], gb_p[:])
            W_sb = []
            for t in range(2):
                w = sb.tile([P, 2 * F], f32r, tag=f"w{t}")
                nc.sync.dma_start(w[:], W_p[P * t:P * (t + 1), :])
                W_sb.append(w)
            hT_sb = []
            for t in range(4):
                ht = sb.tile([P, R], f32r, tag=f"ht{t}")
                nc.sync.dma_start(ht[:], hT_p[P * t:P * (t + 1), :])
                hT_sb.append(ht)
            ones1 = sb.tile([1, P], f32r)
            nc.sync.dma_start(ones1[:], ones1_p[:])
            neg40 = sb.tile([P, 1], f32)
            nc.vector.memset(neg40[:], -BIG2)
            ones64 = sb.tile([F + 1, F], f32)
            nc.vector.memset(ones64[F:F + 1, :], 1.0)

            # ---- adjT shard: evens on sync, odds on gpsimd ----
            adjT_sb = []
            for t in range(NTP):
                at = sb.tile([P, 2, R], fp8, tag=f"adjT{t}")
                nc.sync.dma_start(at[:], adjT_p[P * t:P * (t + 1), :, :])
                adjT_sb.append(at)



            # ---- af tiles (sync: even T, gpsimd: odd T) ----
            af_tiles = {}

            def load_af(g):
                for t in range(NTP):
                    af = aft.tile([P, 2, R], fp8, tag="af", bufs=48,
                                  name=f"af{g}_{t}")
                    q = nc.sync if t < NTP - 2 else nc.scalar
                    q.dma_start(af[:],
                                adjdr_p[P * t:P * (t + 1), :,
                                        R * g:R * (g + 1)])
                    af_tiles[(g, t)] = af

            load_af(0)

            # ---- own-rows preamble: s1 and its broadcast ----
            s1bc = []
            for b in range(2):
                svo = psC.tile([1, R], f32, tag="cnt", name=f"svo{b}")
                for t in range(4):
                    nc.tensor.matmul(svo[:],
                                     V4_sb[t][:, b:b + 1],
                                     hT_sb[t][:],
                                     start=(t == 0), stop=(t == 3))
                sc = sb.tile([1, R], f32r, tag=f"sc{b}")
                nc.vector.tensor_copy(sc[:], svo[:])
                bc = psC.tile([P, R], f32, tag="cnt", name=f"s1bc_ps{b}")
                nc.tensor.matmul(bc[:], ones1[:],
                                 sc[:], start=True, stop=True)
                s1b = sb.tile([P, R], f32, tag=f"s1bc{b}")
                nc.vector.tensor_copy(s1b[:], bc[:])
                s1bc.append(s1b)

            s2_sb = [{}, {}]
            whf_t = [[None] * NT, [None] * NT]

            # ---- attention accumulators (psum rows 0:64 out, 64 sums) ----
            accT = []
            for b in range(2):
                acc_t = psA.tile([F + 1, R], f32, tag=f"acc{b}",
                                 name=f"accT{b}")
                accT.append(acc_t)

            pt_b1 = {}
            pt_b2 = {}

            def chunk_wh(g):
                """Wh^T and s-vectors for k-window g; whf stationaries."""
                hf = []
                for t in range(4):
                    h = sbt.tile([P, R], f32r, tag="hf", bufs=8)
                    nc.scalar.dma_start(
                        h[:], hTf_p[P * t:P * (t + 1), R * g:R * (g + 1)])
                    hf.append(h)
                sv_ps = psC.tile([4, R], f32, tag="cnt", name=f"sv{g}")
                for t in range(4):
                    nc.tensor.matmul(sv_ps[:], V4_sb[t][:],
                                     hf[t][:],
                                     start=(t == 0), stop=(t == 3))
                wh_ps = []
                for b in range(2):
                    whb = psC.tile([F, R], f32, tag="cnt", name=f"wh{g}_{b}")
                    for t in range(2):
                        nc.tensor.matmul(
                            whb[:],
                            W_sb[t][:, F * b:F * (b + 1)],
                            hf[2 * b + t][:],
                            start=(t == 0), stop=(t == 1))
                    wh_ps.append(whb)
                sv_sb = sbt.tile([4, R], f32, tag="svsb", bufs=2)
                nc.vector.tensor_copy(sv_sb[:], sv_ps[:])
                s2d = dram.tile([2, R], f32, name=f"s2d{g}")
                nc.sync.dma_start(s2d[:], sv_sb[2:4, :])
                for b in range(2):
                    wtf = sb.tile([WROWS, R], bf16, tag=f"whTf{b}_{g}")
                    nc.vector.memset(wtf[F:WROWS, :], 0.0)
                    nc.vector.memset(wtf[F:F + 1, :], 1.0)
                    nc.vector.tensor_copy(wtf[0:F, :], wh_ps[b][:])
                    s2g = sb.tile([P, G], f32, tag=f"s2g{b}_{g}")
                    nc.sync.dma_start(
                        s2g[:], s2d[b].rearrange("(q p) -> p q", p=P))
                    s2_sb[b][g] = s2g
                    for j in range(G):
                        kt = G * g + j
                        wf = sb.tile([P, WROWS], bf16, tag=f"whf{b}_{kt}")
                        nc.sync.dma_start(
                            wf[:], wtf[:, P * j:P * (j + 1)],
                            transpose=True)
                        whf_t[b][kt] = wf

            def softmax_b2(g, j, cnt):
                kt = G * g + j
                dt_t = sbt.tile([P, R], bf16, tag="dt", bufs=6)
                nc.sync.dma_start(dt_t[:], dts_p[P * kt:P * (kt + 1), :])
                e2 = sbt.tile([P, R], f32, tag="e", bufs=4)
                nc.scalar.activation(e2[:], s1bc[1][:], Act.Prelu,
                                     bias=s2_sb[1][g][:, j:j + 1],
                                     alpha=ALPHA)
                m = sbt.tile([P, R], f32, tag="m", bufs=6)
                nc.vector.tensor_scalar(m[:], cnt[:], 1.0, BIG2,
                                        op0=Alu.min, op1=Alu.mult)
                nc.vector.tensor_tensor(m[:], m[:], dt_t[:], op=Alu.add)
                nc.vector.tensor_tensor(m[:], m[:], e2[:], op=Alu.add)
                pt = sbt.tile([P, R], bf16, tag="pt", bufs=16)
                nc.scalar.activation(pt[:], m[:], Act.Exp, bias=neg40[:])
                pt_b2[kt] = pt

            def softmax_b1(g, j):
                # fully on DVE (keeps the scalar queue free for exp/hTf)
                kt = G * g + j
                u = sbt.tile([P, R], f32, tag="e", bufs=4)
                nc.vector.tensor_scalar(u[:], s1bc[0][:],
                                        s2_sb[0][g][:, j:j + 1], None,
                                        op0=Alu.add)
                e1 = sbt.tile([P, R], f32, tag="e1", bufs=3)
                nc.vector.scalar_tensor_tensor(e1[:], u[:], ALPHA, u[:],
                                               op0=Alu.mult, op1=Alu.max)
                z = sbt.tile([P, R], f32, tag="m", bufs=6)
                nc.vector.scalar_tensor_tensor(
                    z[:], adjT_sb[kt // 2][:, kt % 2, :], BIG2, e1[:],
                    op0=Alu.mult, op1=Alu.add)
                pt = sbt.tile([P, R], bf16, tag="pt", bufs=16)
                nc.scalar.activation(pt[:], z[:], Act.Exp, bias=neg40[:])
                pt_b1[kt] = pt

            def emit_att(g):
                for j in range(G):
                    kt = G * g + j
                    nc.tensor.matmul(accT[0][:],
                                     whf_t[0][kt][:, 0:F + 1], pt_b1[kt][:],
                                     start=(kt == 0), stop=(kt == NT - 1))
                    nc.tensor.matmul(accT[1][:],
                                     whf_t[1][kt][:, 0:F + 1], pt_b2[kt][:],
                                     start=(kt == 0), stop=(kt == NT - 1))

            # ---- main loop ----
            for g in range(NG):
                chunk_wh(g)
                if g + 1 < NG:
                    load_af(g + 1)
                cnts = [psC.tile([P, R], f32, tag="cnt", name=f"cnt{g}_{j}")
                        for j in range(G)]
                for t in range(NTP):
                    af = af_tiles.pop((g, t))
                    for j in range(G):
                        nc.tensor.matmul(cnts[j][:],
                                         af[:, :, P * j:P * (j + 1)],
                                         adjT_sb[t][:],
                                         perf_mode=DR,
                                         start=(t == 0), stop=(t == NTP - 1))
                for j in range(G):
                    softmax_b2(g, j, cnts[j])
                for j in range(G):
                    softmax_b1(g, j)
                if g >= 1:
                    emit_att(g - 1)
            emit_att(NG - 1)

            # ---- epilogue: normalize, BN stats + AllReduce, BN+lrelu ----
            hp = []
            for b in range(2):
                srec = sb.tile([F + 1, R], f32, tag=f"srec{b}")
                nc.vector.tensor_copy(srec[F:F + 1, :], accT[b][F:F + 1, :])
                rrec = sb.tile([F + 1, R], f32, tag=f"rrec{b}")
                nc.vector.reciprocal(rrec[F:F + 1, :], srec[F:F + 1, :])
                bc_ps = psC.tile([F, R], f32, tag="cnt", name=f"bc_ps{b}")
                nc.tensor.matmul(bc_ps[:], ones64[F:F + 1, :],
                                 rrec[F:F + 1, :],
                                 start=True, stop=True)
                bc_sb = sb.tile([F, R], f32, tag=f"bcs{b}")
                nc.vector.tensor_copy(bc_sb[:], bc_ps[:])
                hp_b = sb.tile([F, R], f32, tag=f"hp{b}")
                nc.vector.tensor_tensor(hp_b[:], accT[b][0:F, :], bc_sb[:],
                                        op=Alu.mult)
                hp.append(hp_b)

            # stats packed [64, 4]: (sum1, sumsq1, sum2, sumsq2)
            sx = sb.tile([F, 4], f32)
            sq = sb.tile([F, R], bf16)
            for b in range(2):
                nc.vector.tensor_reduce(sx[:, 2 * b:2 * b + 1], hp[b][:],
                                        axis=mybir.AxisListType.X,
                                        op=Alu.add)
                nc.scalar.activation(sq[:], hp[b][:], Act.Square,
                                     accum_out=sx[:, 2 * b + 1:2 * b + 2])
            stats_in = dram.tile([F, 4], f32, name="stats_in")
            nc.sync.dma_start(stats_in[:], sx[:])
            stats_out = dram.tile([F, 4], f32, addr_space="Shared",
                                  name="stats_out")
            nc.gpsimd.collective_compute(
                "AllReduce", Alu.add, replica_groups=RG,
                ins=[stats_in[:].opt()], outs=[stats_out[:].opt()])
            gst = sb.tile([F, 4], f32)
            nc.sync.dma_start(gst[:], stats_out[:])

            gst3 = gst[:].rearrange("f (b s) -> f b s", b=2)
            mean = sb.tile([F, 2], f32)
            nc.scalar.mul(mean[:], gst3[:, :, 0], INV_N)
            ex2 = sb.tile([F, 2], f32)
            nc.scalar.mul(ex2[:], gst3[:, :, 1], INV_N)
            var = sb.tile([F, 2], f32)
            nc.vector.scalar_tensor_tensor(var[:], mean[:], -1.0, mean[:],
                                           op0=Alu.mult, op1=Alu.mult)
            nc.vector.tensor_add(var[:], var[:], ex2[:])
            nc.vector.tensor_scalar_add(var[:], var[:], EPS)
            std = sb.tile([F, 2], f32)
            nc.scalar.activation(std[:], var[:], Act.Sqrt)
            rstd = sb.tile([F, 2], f32)
            nc.vector.reciprocal(rstd[:], std[:])
            gb3 = gb_sb[:].rearrange("f (b s) -> f b s", b=2)
            scale = sb.tile([F, 2], f32)
            nc.vector.tensor_mul(scale[:], gb3[:, :, 0], rstd[:])
            nbias = sb.tile([F, 2], f32)
            nc.vector.scalar_tensor_tensor(nbias[:], mean[:], -1.0, scale[:],
                                           op0=Alu.mult, op1=Alu.mult)
            nc.vector.tensor_add(nbias[:], nbias[:], gb3[:, :, 1])

            # fused BN apply + lrelu; transpose out per branch
            ob = sb.tile([P, RQ, 2 * F], f32)
            for b in range(2):
                finb = sb.tile([F, R], f32, tag=f"fin{b}")
                nc.scalar.activation(finb[:], hp[b][:], Act.Prelu,
                                     bias=nbias[:, b:b + 1],
                                     scale=scale[:, b:b + 1], alpha=ALPHA)
                for q in range(RQ):
                    tp = psC.tile([P, F], f32, tag="cnt", name=f"otp{b}_{q}")
                    nc.tensor.transpose(tp[:], finb[:, P * q:P * (q + 1)],
                                        ident[0:F, 0:F])
                    nc.vector.tensor_copy(ob[:, q, F * b:F * (b + 1)],
                                          tp[:])
            nc.sync.dma_start(
                out_p.rearrange("(q p) f -> p q f", p=P), ob[:])

    nc.compile()
    return nc


def _get_nc():
    if "nc" not in _CACHED:
        _CACHED["nc"] = build_nc()
    return _CACHED["nc"]


def make_in_maps(h, adj, W1, W2, a, gamma, beta):
    h = np.asarray(h, dtype=np.float32)
    adj = np.asarray(adj, dtype=np.float32)
    W1 = np.asarray(W1, np.float32)
    W2 = np.asarray(W2, np.float32)
    W12 = np.concatenate([W1, W2], axis=1)
    a_flat = np.asarray(a, np.float32).reshape(2 * F)
    a1, a2 = a_flat[:F], a_flat[F:]
    # V4 = folded W @ a vectors: s = h @ V4 gives s1/s2 for both branches
    V4 = np.zeros((IN_F, 4), dtype=np.float32)
    V4[:HALF, 0] = W1 @ a1
    V4[HALF:, 1] = W2 @ a1
    V4[:HALF, 2] = W1 @ a2
    V4[HALF:, 3] = W2 @ a2
    gamma = np.asarray(gamma, np.float32)
    beta = np.asarray(beta, np.float32)
    gb = np.stack([gamma[:F], beta[:F], gamma[F:], beta[F:]], axis=1)
    ident = np.eye(P, dtype=np.float32)

    fp8 = ml_dtypes.float8_e4m3fn
    adj_f8 = adj.astype(fp8)
    # full adj in DoubleRow layout: adjdr[128T+p, s, k] = adj[256T+128s+p, k]
    adjdr = np.ascontiguousarray(
        adj_f8.reshape(NTP, 2, P, N).transpose(0, 2, 1, 3)
        .reshape(NTP * P, 2, N))
    hTf = np.ascontiguousarray(h.T)

    in_maps = []
    for c in range(M_CORES):
        r0 = c * R
        shT = np.ascontiguousarray(adj[r0:r0 + R, :].T).astype(fp8)
        adjT = np.ascontiguousarray(
            shT.reshape(NTP, 2, P, R).transpose(0, 2, 1, 3)
            .reshape(NTP * P, 2, R))
        dts = np.zeros((N, R), dtype=ml_dtypes.bfloat16)
        dts[np.arange(r0, r0 + R), np.arange(R)] = -BIG2
        in_maps.append({
            "hT": np.ascontiguousarray(h[r0:r0 + R, :].T),
            "hTf": hTf,
            "adjdr": adjdr,
            "adjT": adjT,
            "dts": dts,
            "W12": W12,
            "V4": V4,
            "gb": gb,
            "ident": ident,
            "ones1r": np.ones((1, P), dtype=np.float32),
        })
    return in_maps


def kernel(h, adj, W1, W2, a, gamma, beta):
    from concourse.bass_utils import run_bass_kernel_spmd

    in_maps = make_in_maps(h, adj, W1, W2, a, gamma, beta)
    nc = _get_nc()
    res = run_bass_kernel_spmd(nc, in_maps, core_ids=list(range(M_CORES)))
    outs = [np.asarray(res.results[c]["out"]) for c in range(M_CORES)]
    return np.concatenate(outs, axis=0)


# revision 35
# speedup vs baseline: 1.3784x; 1.3238x over previous
"""Distributed Bass kernel for nn_AttentionLayer (2-branch GAT-style layer).

Row-shard over 8 NeuronCores (512 rows each), transposed on-chip layout
(k on partitions, own-row i on free axis) so masked softmax feeds the PE
attention matmuls without transposes.

v3 design:
- Full adj AND full h^T are REPLICATED to every core in HBM.  No mid-
  kernel collectives at all (the first collective's implicit device
  barrier costs ~76us and serializes the CC stream); a dummy 16-byte
  AllGather is fired at t=0 so the barrier overlaps the whole kernel and
  the final BN-stats AllReduce launches instantly.
- Wh^T is computed per 512-column chunk with float32r matmuls (1 cyc/row)
  and cast to bf16 rows [Wh^T | ones]; per-k-tile stationaries
  [128, 80] for the attention matmul come from XBAR DMA transposes
  (zero PE cost).  The ones row makes each attention matmul also emit
  the softmax denominator (psum row 64).
- s1/s2 come from V = W @ a folded on the host: s = h @ V, computed in
  f32r on PE (exact enough), so softmax starts ~15us into the kernel.
- lrelu fused via Prelu (alpha=0.2) which shares the activation table
  set with Exp/Square (no table reloads); masking via the +-40 bias
  trick with -40 as the Exp activation bias.
- adj2 counts (2-hop) on PE in fp8 DoubleRow, exact in f32 psum;
  attention matmuls run one group behind (software pipelining).
"""

import sys
import numpy as np

for _p in ("/opt/trn_rl_repo", "/opt/trn_rl_repo/concourse"):
    if _p not in sys.path:
        sys.path.insert(0, _p)

import ml_dtypes

N = 4096
M_CORES = 8
R = N // M_CORES          # 512 rows per core
IN_F = 512
HALF = IN_F // 2          # 256
F = 64
P = 128                   # partitions
NT = N // P               # 32 k tiles
NTP = NT // 2             # 16 DoubleRow k-tile pairs
G = 4                     # k-tiles per psum group
NG = NT // G              # 8 groups
WROWS = 80                # whTf rows incl ones row + pad (mult of 16)
ALPHA = 0.2
EPS = 1e-5
BIG2 = 40.0               # mask bias; exp(e - 40) ~ 0 for e <= ~12
INV_N = 1.0 / N

_CACHED = {}


def build_nc():
    from concourse import bacc, tile, mybir

    f32 = mybir.dt.float32
    f32r = mybir.dt.float32r
    bf16 = mybir.dt.bfloat16
    fp8 = mybir.dt.float8e4
    Alu = mybir.AluOpType
    Act = mybir.ActivationFunctionType
    DR = mybir.MatmulPerfMode.DoubleRow

    nc = bacc.Bacc("TRN2", target_bir_lowering=False, debug=False,
                   num_devices=M_CORES)

    hT_p = nc.declare_dram_parameter("hT", [IN_F, R], f32r, isOutput=False)
    hTf_p = nc.declare_dram_parameter("hTf", [IN_F, N], bf16, isOutput=False)
    adjdr_p = nc.declare_dram_parameter("adjdr", [NTP * P, 2, N], fp8,
                                        isOutput=False)
    adjT_p = nc.declare_dram_parameter("adjT", [NTP * P, 2, R], fp8,
                                       isOutput=False)
    dts_p = nc.declare_dram_parameter("dts", [N, R], bf16, isOutput=False)
    WV_p = nc.declare_dram_parameter("WV", [IN_F, F + 1], bf16,
                                     isOutput=False)
    V4_p = nc.declare_dram_parameter("V4", [IN_F, 4], f32r, isOutput=False)
    gb_p = nc.declare_dram_parameter("gb", [F, 4], f32, isOutput=False)
    id_p = nc.declare_dram_parameter("ident", [P, P], f32, isOutput=False)
    ones1_p = nc.declare_dram_parameter("ones1r", [1, P], f32r,
                                        isOutput=False)
    out_p = nc.declare_dram_parameter("out", [R, 2 * F], f32, isOutput=True)

    RG = [list(range(M_CORES))]
    RQ = R // P               # 4 row blocks per core

    with tile.TileContext(nc) as tc:
        with (
            tc.tile_pool(name="sb", bufs=1) as sb,
            tc.tile_pool(name="aft", bufs=1) as aft,
            tc.tile_pool(name="sbt", bufs=3) as sbt,
            tc.tile_pool(name="psA", bufs=1, space="PSUM") as psA,
            tc.tile_pool(name="psC", bufs=6, space="PSUM") as psC,
            tc.tile_pool(name="dram", bufs=1, space="DRAM") as dram,
        ):
            # ---- small persistent loads (sync queue) ----
            ident = sb.tile([P, P], f32)
            nc.sync.dma_start(ident[:], id_p[:])
            V4_sb = []
            for t in range(4):
                v = sb.tile([P, 4], f32r, tag=f"v4_{t}")
                nc.sync.dma_start(v[:], V4_p[P * t:P * (t + 1), :])
                V4_sb.append(v)
            gb_sb = sb.tile([F, 4], f32)
            nc.sync.dma_start(gb_sb[:], gb_p[:])
            WV_sb = []
            for t in range(4):
                wv = sb.tile([P, F + 1], bf16, tag=f"wv{t}")
                nc.sync.dma_start(wv[:], WV_p[P * t:P * (t + 1), :])
                WV_sb.append(wv)
            hT_sb = []
            for t in range(4):
                ht = sb.tile([P, R], f32r, tag=f"ht{t}")
                nc.sync.dma_start(ht[:], hT_p[P * t:P * (t + 1), :])
                hT_sb.append(ht)
            ones1 = sb.tile([1, P], f32r)
            nc.sync.dma_start(ones1[:], ones1_p[:])
            neg40 = sb.tile([P, 1], f32)
            nc.vector.memset(neg40[:], -BIG2)
            ones64 = sb.tile([F + 1, F], f32)
            nc.vector.memset(ones64[F:F + 1, :], 1.0)

            # ---- adjT shard: evens on sync, odds on gpsimd ----
            adjT_sb = []
            for t in range(NTP):
                at = sb.tile([P, 2, R], fp8, tag=f"adjT{t}")
                nc.sync.dma_start(at[:], adjT_p[P * t:P * (t + 1), :, :])
                adjT_sb.append(at)



            # ---- af tiles (sync: even T, gpsimd: odd T) ----
            af_tiles = {}

            def load_af(g):
                for t in range(NTP):
                    af = aft.tile([P, 2, R], fp8, tag="af", bufs=48,
                                  name=f"af{g}_{t}")
                    q = nc.sync if t < NTP - 2 else nc.scalar
                    q.dma_start(af[:],
                                adjdr_p[P * t:P * (t + 1), :,
                                        R * g:R * (g + 1)])
                    af_tiles[(g, t)] = af

            load_af(0)

            # warmup collective: absorbs the one-time device barrier + CC
            # stream setup while local compute proceeds.  (collectives
            # cannot read IO tensors -> bounce via sbuf)
            warm_sb = sb.tile([1, 4], f32)
            nc.vector.memset(warm_sb[:], 0.0)
            warm_in = dram.tile([1, 4], f32, name="warm_in")
            nc.gpsimd.dma_start(warm_in[:], warm_sb[:])
            dummy_out = dram.tile([M_CORES, 4], f32, addr_space="Shared",
                                  name="dummy_out")
            nc.gpsimd.collective_compute(
                "AllGather", Alu.bypass, replica_groups=RG,
                ins=[warm_in[:].opt()], outs=[dummy_out[:].opt()])

            # ---- own-rows preamble: s1 and its broadcast ----
            s1bc = []
            for b in range(2):
                svo = psC.tile([1, R], f32, tag="cnt", name=f"svo{b}")
                for t in range(4):
                    nc.tensor.matmul(svo[:],
                                     V4_sb[t][:, b:b + 1],
                                     hT_sb[t][:],
                                     start=(t == 0), stop=(t == 3))
                sc = sb.tile([1, R], f32r, tag=f"sc{b}")
                nc.vector.tensor_copy(sc[:], svo[:])
                bc = psC.tile([P, R], f32, tag="cnt", name=f"s1bc_ps{b}")
                nc.tensor.matmul(bc[:], ones1[:],
                                 sc[:], start=True, stop=True)
                s1b = sb.tile([P, R], f32, tag=f"s1bc{b}")
                nc.vector.tensor_copy(s1b[:], bc[:])
                s1bc.append(s1b)

            whf_t = [[], []]
            svn_sb = []
            for kt in range(NT):
                sv_t = sb.tile([P, 2], f32, tag=f"svn{kt}", name=f"svn{kt}")
                svn_sb.append(sv_t)
                for b in range(2):
                    wf = sb.tile([P, F + 1], bf16, tag=f"wf{b}_{kt}",
                                 name=f"wf{b}_{kt}")
                    nc.vector.memset(wf[:, F:F + 1], 1.0)
                    whf_t[b].append(wf)

            # ---- attention accumulators (psum rows 0:64 out, 64 sums) ----
            accT = []
            for b in range(2):
                acc_t = psA.tile([F + 1, R], f32, tag=f"acc{b}",
                                 name=f"accT{b}")
                accT.append(acc_t)

            pt_b1 = {}
            pt_b2 = {}

            def chunk_wh(g):
                """Natural-layout Wh tiles + s2 bias for k-window g."""
                hf = []
                for t in range(4):
                    h = sbt.tile([P, R], bf16, tag="hf", bufs=8)
                    nc.scalar.dma_start(
                        h[:], hTf_p[P * t:P * (t + 1), R * g:R * (g + 1)])
                    hf.append(h)
                whn = []
                for b in range(2):
                    wb = psC.tile([P, G, F + 1], f32, tag="cnt",
                                  name=f"whn{g}_{b}")
                    for j in range(G):
                        for t2 in range(2):
                            nc.tensor.matmul(
                                wb[:, j, :],
                                hf[2 * b + t2][:, P * j:P * (j + 1)],
                                WV_sb[2 * b + t2][:],
                                start=(t2 == 0), stop=(t2 == 1))
                    whn.append(wb)
                for j in range(G):
                    kt = G * g + j
                    for b in range(2):
                        nc.vector.tensor_copy(svn_sb[kt][:, b:b + 1],
                                              whn[b][:, j, F:F + 1])
                        nc.vector.tensor_copy(whf_t[b][kt][:, 0:F],
                                              whn[b][:, j, 0:F])

            def softmax_b2(g, j, cnt):
                kt = G * g + j
                dt_t = sbt.tile([P, R], bf16, tag="dt", bufs=6)
                nc.sync.dma_start(dt_t[:], dts_p[P * kt:P * (kt + 1), :])
                e2 = sbt.tile([P, R], f32, tag="e", bufs=4)
                nc.scalar.activation(e2[:], s1bc[1][:], Act.Prelu,
                                     bias=svn_sb[kt][:, 1:2],
                                     alpha=ALPHA)
                m = sbt.tile([P, R], f32, tag="m", bufs=6)
                nc.vector.tensor_scalar(m[:], cnt[:], 1.0, BIG2,
                                        op0=Alu.min, op1=Alu.mult)
                nc.vector.tensor_tensor(m[:], m[:], dt_t[:], op=Alu.add)
                nc.vector.tensor_tensor(m[:], m[:], e2[:], op=Alu.add)
                pt = sbt.tile([P, R], bf16, tag="pt", bufs=16)
                nc.scalar.activation(pt[:], m[:], Act.Exp, bias=neg40[:])
                pt_b2[kt] = pt

            def softmax_b1(g, j):
                # fully on DVE (keeps the scalar queue free for exp/hTf)
                kt = G * g + j
                u = sbt.tile([P, R], f32, tag="e", bufs=4)
                nc.vector.tensor_scalar(u[:], s1bc[0][:],
                                        svn_sb[kt][:, 0:1], None,
                                        op0=Alu.add)
                e1 = sbt.tile([P, R], f32, tag="e1", bufs=4)
                nc.vector.scalar_tensor_tensor(e1[:], u[:], ALPHA, u[:],
                                               op0=Alu.mult, op1=Alu.max)
                z = sbt.tile([P, R], f32, tag="m", bufs=6)
                nc.vector.scalar_tensor_tensor(
                    z[:], adjT_sb[kt // 2][:, kt % 2, :], BIG2, e1[:],
                    op0=Alu.mult, op1=Alu.add)
                pt = sbt.tile([P, R], bf16, tag="pt", bufs=16)
                nc.scalar.activation(pt[:], z[:], Act.Exp, bias=neg40[:])
                pt_b1[kt] = pt

            def emit_att(g):
                for j in range(G):
                    kt = G * g + j
                    nc.tensor.matmul(accT[0][:],
                                     whf_t[0][kt][:, 0:F + 1], pt_b1[kt][:],
                                     start=(kt == 0), stop=(kt == NT - 1))
                    nc.tensor.matmul(accT[1][:],
                                     whf_t[1][kt][:, 0:F + 1], pt_b2[kt][:],
                                     start=(kt == 0), stop=(kt == NT - 1))

            # ---- main loop ----
            for g in range(NG):
                chunk_wh(g)
                if g + 1 < NG:
                    load_af(g + 1)
                cnts = [psC.tile([P, R], f32, tag="cnt", name=f"cnt{g}_{j}")
                        for j in range(G)]
                for t in range(NTP):
                    af = af_tiles.pop((g, t))
                    for j in range(G):
                        nc.tensor.matmul(cnts[j][:],
                                         af[:, :, P * j:P * (j + 1)],
                                         adjT_sb[t][:],
                                         perf_mode=DR,
                                         start=(t == 0), stop=(t == NTP - 1))
                for j in range(G):
                    softmax_b2(g, j, cnts[j])
                for j in range(G):
                    softmax_b1(g, j)
                if g >= 1:
                    emit_att(g - 1)
            emit_att(NG - 1)

            # ---- epilogue: normalize, BN stats + AllReduce, BN+lrelu ----
            hp = []
            for b in range(2):
                srec = sb.tile([F + 1, R], f32, tag=f"srec{b}")
                nc.vector.tensor_copy(srec[F:F + 1, :], accT[b][F:F + 1, :])
                rrec = sb.tile([F + 1, R], f32, tag=f"rrec{b}")
                nc.vector.reciprocal(rrec[F:F + 1, :], srec[F:F + 1, :])
                bc_ps = psC.tile([F, R], f32, tag="cnt", name=f"bc_ps{b}")
                nc.tensor.matmul(bc_ps[:], ones64[F:F + 1, :],
                                 rrec[F:F + 1, :],
                                 start=True, stop=True)
                bc_sb = sb.tile([F, R], f32, tag=f"bcs{b}")
                nc.vector.tensor_copy(bc_sb[:], bc_ps[:])
                hp_b = sb.tile([F, R], f32, tag=f"hp{b}")
                nc.vector.tensor_tensor(hp_b[:], accT[b][0:F, :], bc_sb[:],
                                        op=Alu.mult)
                hp.append(hp_b)

            # stats packed [64, 4]: (sum1, sumsq1, sum2, sumsq2)
            sx = sb.tile([F, 4], f32)
            sq = sb.tile([F, R], bf16)
            for b in range(2):
                nc.vector.tensor_reduce(sx[:, 2 * b:2 * b + 1], hp[b][:],
                                        axis=mybir.AxisListType.X,
                                        op=Alu.add)
                nc.scalar.activation(sq[:], hp[b][:], Act.Square,
                                     accum_out=sx[:, 2 * b + 1:2 * b + 2])
            stats_in = dram.tile([F, 4], f32, name="stats_in")
            nc.sync.dma_start(stats_in[:], sx[:])
            stats_out = dram.tile([F, 4], f32, addr_space="Shared",
                                  name="stats_out")
            nc.gpsimd.collective_compute(
                "AllReduce", Alu.add, replica_groups=RG,
                ins=[stats_in[:].opt()], outs=[stats_out[:].opt()])
            gst = sb.tile([F, 4], f32)
            nc.sync.dma_start(gst[:], stats_out[:])

            gst3 = gst[:].rearrange("f (b s) -> f b s", b=2)
            mean = sb.tile([F, 2], f32)
            nc.scalar.mul(mean[:], gst3[:, :, 0], INV_N)
            ex2 = sb.tile([F, 2], f32)
            nc.scalar.mul(ex2[:], gst3[:, :, 1], INV_N)
            var = sb.tile([F, 2], f32)
            nc.vector.scalar_tensor_tensor(var[:], mean[:], -1.0, mean[:],
                                           op0=Alu.mult, op1=Alu.mult)
            nc.vector.tensor_add(var[:], var[:], ex2[:])
            nc.vector.tensor_scalar_add(var[:], var[:], EPS)
            std = sb.tile([F, 2], f32)
            nc.scalar.activation(std[:], var[:], Act.Sqrt)
            rstd = sb.tile([F, 2], f32)
            nc.vector.reciprocal(rstd[:], std[:])
            gb3 = gb_sb[:].rearrange("f (b s) -> f b s", b=2)
            scale = sb.tile([F, 2], f32)
            nc.vector.tensor_mul(scale[:], gb3[:, :, 0], rstd[:])
            nbias = sb.tile([F, 2], f32)
            nc.vector.scalar_tensor_tensor(nbias[:], mean[:], -1.0, scale[:],
                                           op0=Alu.mult, op1=Alu.mult)
            nc.vector.tensor_add(nbias[:], nbias[:], gb3[:, :, 1])

            # fused BN apply + lrelu; transpose out per branch
            ob = sb.tile([P, RQ, 2 * F], f32)
            for b in range(2):
                finb = sb.tile([F, R], f32, tag=f"fin{b}")
                nc.scalar.activation(finb[:], hp[b][:], Act.Prelu,
                                     bias=nbias[:, b:b + 1],
                                     scale=scale[:, b:b + 1], alpha=ALPHA)
                for q in range(RQ):
                    tp = psC.tile([P, F], f32, tag="cnt", name=f"otp{b}_{q}")
                    nc.tensor.transpose(tp[:], finb[:, P * q:P * (q + 1)],
                                        ident[0:F, 0:F])
                    nc.vector.tensor_copy(ob[:, q, F * b:F * (b + 1)],
                                          tp[:])
            nc.sync.dma_start(
                out_p.rearrange("(q p) f -> p q f", p=P), ob[:])

    nc.compile()
    return nc


def _get_nc():
    if "nc" not in _CACHED:
        _CACHED["nc"] = build_nc()
    return _CACHED["nc"]


def make_in_maps(h, adj, W1, W2, a, gamma, beta):
    h = np.asarray(h, dtype=np.float32)
    adj = np.asarray(adj, dtype=np.float32)
    W1 = np.asarray(W1, np.float32)
    W2 = np.asarray(W2, np.float32)
    a_flat = np.asarray(a, np.float32).reshape(2 * F)
    a1, a2 = a_flat[:F], a_flat[F:]
    # V4 = folded W @ a vectors: s = h @ V4 gives s1/s2 for both branches
    V4 = np.zeros((IN_F, 4), dtype=np.float32)
    V4[:HALF, 0] = W1 @ a1
    V4[HALF:, 1] = W2 @ a1
    V4[:HALF, 2] = W1 @ a2
    V4[HALF:, 3] = W2 @ a2
    # WV [512, 65]: rows 128t:128(t+1) -> branch b = t//2: cols 0:64 the
    # W block for that feature tile, col 64 = folded (W @ a2) weights so
    # the natural-Wh matmul also emits the s2 bias per k
    WV = np.zeros((IN_F, F + 1), dtype=ml_dtypes.bfloat16)
    WV[:HALF, 0:F] = W1
    WV[HALF:, 0:F] = W2
    WV[:, F] = V4[:, 2] + V4[:, 3]
    gamma = np.asarray(gamma, np.float32)
    beta = np.asarray(beta, np.float32)
    gb = np.stack([gamma[:F], beta[:F], gamma[F:], beta[F:]], axis=1)
    ident = np.eye(P, dtype=np.float32)

    fp8 = ml_dtypes.float8_e4m3fn
    adj_f8 = adj.astype(fp8)
    # full adj in DoubleRow layout: adjdr[128T+p, s, k] = adj[256T+128s+p, k]
    adjdr = np.ascontiguousarray(
        adj_f8.reshape(NTP, 2, P, N).transpose(0, 2, 1, 3)
        .reshape(NTP * P, 2, N))
    hTf = np.ascontiguousarray(h.T).astype(ml_dtypes.bfloat16)

    in_maps = []
    for c in range(M_CORES):
        r0 = c * R
        shT = np.ascontiguousarray(adj[r0:r0 + R, :].T).astype(fp8)
        adjT = np.ascontiguousarray(
            shT.reshape(NTP, 2, P, R).transpose(0, 2, 1, 3)
            .reshape(NTP * P, 2, R))
        dts = np.zeros((N, R), dtype=ml_dtypes.bfloat16)
        dts[np.arange(r0, r0 + R), np.arange(R)] = -BIG2
        in_maps.append({
            "hT": np.ascontiguousarray(h[r0:r0 + R, :].T),
            "hTf": hTf,
            "adjdr": adjdr,
            "adjT": adjT,
            "dts": dts,
            "WV": WV,
            "V4": V4,
            "gb": gb,
            "ident": ident,
            "ones1r": np.ones((1, P), dtype=np.float32),
        })
    return in_maps


def kernel(h, adj, W1, W2, a, gamma, beta):
    from concourse.bass_utils import run_bass_kernel_spmd

    in_maps = make_in_maps(h, adj, W1, W2, a, gamma, beta)
    nc = _get_nc()
    res = run_bass_kernel_spmd(nc, in_maps, core_ids=list(range(M_CORES)))
    outs = [np.asarray(res.results[c]["out"]) for c in range(M_CORES)]
    return np.concatenate(outs, axis=0)


# revision 36
# speedup vs baseline: 1.6141x; 1.1710x over previous
"""Distributed Bass kernel for nn_AttentionLayer (2-branch GAT-style layer).

Row-shard over 8 NeuronCores (512 rows each), transposed on-chip layout
(k on partitions, own-row i on free axis) so masked softmax feeds the PE
attention matmuls without transposes.

v3 design:
- Full adj AND full h^T are REPLICATED to every core in HBM.  No mid-
  kernel collectives at all (the first collective's implicit device
  barrier costs ~76us and serializes the CC stream); a dummy 16-byte
  AllGather is fired at t=0 so the barrier overlaps the whole kernel and
  the final BN-stats AllReduce launches instantly.
- Wh^T is computed per 512-column chunk with float32r matmuls (1 cyc/row)
  and cast to bf16 rows [Wh^T | ones]; per-k-tile stationaries
  [128, 80] for the attention matmul come from XBAR DMA transposes
  (zero PE cost).  The ones row makes each attention matmul also emit
  the softmax denominator (psum row 64).
- s1/s2 come from V = W @ a folded on the host: s = h @ V, computed in
  f32r on PE (exact enough), so softmax starts ~15us into the kernel.
- lrelu fused via Prelu (alpha=0.2) which shares the activation table
  set with Exp/Square (no table reloads); masking via the +-40 bias
  trick with -40 as the Exp activation bias.
- adj2 counts (2-hop) on PE in fp8 DoubleRow, exact in f32 psum;
  attention matmuls run one group behind (software pipelining).
"""

import sys
import numpy as np

for _p in ("/opt/trn_rl_repo", "/opt/trn_rl_repo/concourse"):
    if _p not in sys.path:
        sys.path.insert(0, _p)

import ml_dtypes

N = 4096
M_CORES = 8
R = N // M_CORES          # 512 rows per core
IN_F = 512
HALF = IN_F // 2          # 256
F = 64
P = 128                   # partitions
NT = N // P               # 32 k tiles
NTP = NT // 2             # 16 DoubleRow k-tile pairs
G = 4                     # k-tiles per psum group
NG = NT // G              # 8 groups
WROWS = 80                # whTf rows incl ones row + pad (mult of 16)
ALPHA = 0.2
EPS = 1e-5
BIG2 = 40.0               # mask bias; exp(e - 40) ~ 0 for e <= ~12
INV_N = 1.0 / N

_CACHED = {}


def build_nc():
    from concourse import bacc, tile, mybir

    f32 = mybir.dt.float32
    f32r = mybir.dt.float32r
    bf16 = mybir.dt.bfloat16
    fp8 = mybir.dt.float8e4
    Alu = mybir.AluOpType
    Act = mybir.ActivationFunctionType
    DR = mybir.MatmulPerfMode.DoubleRow

    nc = bacc.Bacc("TRN2", target_bir_lowering=False, debug=False,
                   num_devices=M_CORES)

    hT_p = nc.declare_dram_parameter("hT", [IN_F, R], f32r, isOutput=False)
    hTf_p = nc.declare_dram_parameter("hTf", [IN_F, N], bf16, isOutput=False)
    adjdr_p = nc.declare_dram_parameter("adjdr", [NTP * P, 2, N], fp8,
                                        isOutput=False)
    adjT_p = nc.declare_dram_parameter("adjT", [NTP * P, 2, R], fp8,
                                       isOutput=False)
    dts_p = nc.declare_dram_parameter("dts", [N, R], bf16, isOutput=False)
    WV_p = nc.declare_dram_parameter("WV", [IN_F, F + 1], bf16,
                                     isOutput=False)
    V4_p = nc.declare_dram_parameter("V4", [IN_F, 4], f32r, isOutput=False)
    gb_p = nc.declare_dram_parameter("gb", [F, 4], f32, isOutput=False)
    id_p = nc.declare_dram_parameter("ident", [P, P], f32, isOutput=False)
    ones1_p = nc.declare_dram_parameter("ones1r", [1, P], f32r,
                                        isOutput=False)
    out_p = nc.declare_dram_parameter("out", [R, 2 * F], f32, isOutput=True)

    RG = [list(range(M_CORES))]
    RQ = R // P               # 4 row blocks per core

    with tile.TileContext(nc) as tc:
        with (
            tc.tile_pool(name="sb", bufs=1) as sb,
            tc.tile_pool(name="aft", bufs=1) as aft,
            tc.tile_pool(name="sbt", bufs=3) as sbt,
            tc.tile_pool(name="psA", bufs=1, space="PSUM") as psA,
            tc.tile_pool(name="psC", bufs=6, space="PSUM") as psC,
            tc.tile_pool(name="dram", bufs=1, space="DRAM") as dram,
        ):
            # ---- small persistent loads (sync queue) ----
            ident = sb.tile([P, P], f32)
            nc.sync.dma_start(ident[:], id_p[:])
            V4_sb = []
            for t in range(4):
                v = sb.tile([P, 4], f32r, tag=f"v4_{t}")
                nc.sync.dma_start(v[:], V4_p[P * t:P * (t + 1), :])
                V4_sb.append(v)
            gb_sb = sb.tile([F, 4], f32)
            nc.sync.dma_start(gb_sb[:], gb_p[:])
            WV_sb = []
            for t in range(4):
                wv = sb.tile([P, F + 1], bf16, tag=f"wv{t}")
                nc.sync.dma_start(wv[:], WV_p[P * t:P * (t + 1), :])
                WV_sb.append(wv)
            hT_sb = []
            for t in range(4):
                ht = sb.tile([P, R], f32r, tag=f"ht{t}")
                nc.sync.dma_start(ht[:], hT_p[P * t:P * (t + 1), :])
                hT_sb.append(ht)
            ones1 = sb.tile([1, P], f32r)
            nc.sync.dma_start(ones1[:], ones1_p[:])
            neg40 = sb.tile([P, 1], f32)
            nc.vector.memset(neg40[:], -BIG2)
            ones64 = sb.tile([F + 1, F], f32)
            nc.vector.memset(ones64[F:F + 1, :], 1.0)

            # ---- adjT shard: evens on sync, odds on gpsimd ----
            adjT_sb = []
            for t in range(NTP):
                at = sb.tile([P, 2, R], fp8, tag=f"adjT{t}")
                nc.gpsimd.dma_start(at[:], adjT_p[P * t:P * (t + 1), :, :])
                adjT_sb.append(at)



            # ---- af tiles (sync: even T, gpsimd: odd T) ----
            af_tiles = {}

            def load_af(g):
                for t in range(NTP):
                    af = aft.tile([P, 2, R], fp8, tag="af", bufs=48,
                                  name=f"af{g}_{t}")
                    q = nc.sync if t < NTP - 2 else nc.scalar
                    q.dma_start(af[:],
                                adjdr_p[P * t:P * (t + 1), :,
                                        R * g:R * (g + 1)])
                    af_tiles[(g, t)] = af

            load_af(0)

            # warmup collective: absorbs the one-time device barrier + CC
            # stream setup while local compute proceeds.  (collectives
            # cannot read IO tensors -> bounce via sbuf)
            warm_sb = sb.tile([1, 4], f32)
            nc.vector.memset(warm_sb[:], 0.0)
            warm_in = dram.tile([1, 4], f32, name="warm_in")
            nc.gpsimd.dma_start(warm_in[:], warm_sb[:])
            dummy_out = dram.tile([M_CORES, 4], f32, addr_space="Shared",
                                  name="dummy_out")
            nc.gpsimd.collective_compute(
                "AllGather", Alu.bypass, replica_groups=RG,
                ins=[warm_in[:].opt()], outs=[dummy_out[:].opt()])

            # ---- own-rows preamble: s1 and its broadcast ----
            s1bc = []
            for b in range(2):
                svo = psC.tile([1, R], f32, tag="cnt", name=f"svo{b}")
                for t in range(4):
                    nc.tensor.matmul(svo[:],
                                     V4_sb[t][:, b:b + 1],
                                     hT_sb[t][:],
                                     start=(t == 0), stop=(t == 3))
                sc = sb.tile([1, R], f32r, tag=f"sc{b}")
                nc.vector.tensor_copy(sc[:], svo[:])
                bc = psC.tile([P, R], f32, tag="cnt", name=f"s1bc_ps{b}")
                nc.tensor.matmul(bc[:], ones1[:],
                                 sc[:], start=True, stop=True)
                s1b = sb.tile([P, R], f32, tag=f"s1bc{b}")
                nc.vector.tensor_copy(s1b[:], bc[:])
                s1bc.append(s1b)

            whf_t = [[], []]
            svn_sb = []
            for kt in range(NT):
                sv_t = sb.tile([P, 2], f32, tag=f"svn{kt}", name=f"svn{kt}")
                svn_sb.append(sv_t)
                for b in range(2):
                    wf = sb.tile([P, F + 1], bf16, tag=f"wf{b}_{kt}",
                                 name=f"wf{b}_{kt}")
                    nc.vector.memset(wf[:, F:F + 1], 1.0)
                    whf_t[b].append(wf)

            # ---- attention accumulators (psum rows 0:64 out, 64 sums) ----
            accT = []
            for b in range(2):
                acc_t = psA.tile([F + 1, R], f32, tag=f"acc{b}",
                                 name=f"accT{b}")
                accT.append(acc_t)

            pt_b1 = {}
            pt_b2 = {}

            def chunk_wh(g):
                """Natural-layout Wh tiles + s2 bias for k-window g."""
                hf = []
                for t in range(4):
                    h = sbt.tile([P, R], bf16, tag="hf", bufs=8)
                    nc.scalar.dma_start(
                        h[:], hTf_p[P * t:P * (t + 1), R * g:R * (g + 1)])
                    hf.append(h)
                whn = []
                for b in range(2):
                    wb = psC.tile([P, G, F + 1], f32, tag="cnt",
                                  name=f"whn{g}_{b}")
                    for j in range(G):
                        for t2 in range(2):
                            nc.tensor.matmul(
                                wb[:, j, :],
                                hf[2 * b + t2][:, P * j:P * (j + 1)],
                                WV_sb[2 * b + t2][:],
                                start=(t2 == 0), stop=(t2 == 1))
                    whn.append(wb)
                for j in range(G):
                    kt = G * g + j
                    for b in range(2):
                        nc.vector.tensor_copy(svn_sb[kt][:, b:b + 1],
                                              whn[b][:, j, F:F + 1])
                        nc.vector.tensor_copy(whf_t[b][kt][:, 0:F],
                                              whn[b][:, j, 0:F])

            def softmax_b2(g, j, cnt):
                kt = G * g + j
                dt_t = sbt.tile([P, R], bf16, tag="dt", bufs=6)
                nc.sync.dma_start(dt_t[:], dts_p[P * kt:P * (kt + 1), :])
                e2 = sbt.tile([P, R], f32, tag="e", bufs=4)
                nc.scalar.activation(e2[:], s1bc[1][:], Act.Prelu,
                                     bias=svn_sb[kt][:, 1:2],
                                     alpha=ALPHA)
                q = sbt.tile([P, R], f32, tag="q", bufs=4)
                nc.vector.scalar_tensor_tensor(q[:], cnt[:], 1.0, dt_t[:],
                                               op0=Alu.min, op1=Alu.add)
                m = sbt.tile([P, R], f32, tag="m", bufs=6)
                nc.vector.scalar_tensor_tensor(m[:], q[:], BIG2, e2[:],
                                               op0=Alu.mult, op1=Alu.add)
                pt = sbt.tile([P, R], bf16, tag="pt", bufs=16)
                nc.scalar.activation(pt[:], m[:], Act.Exp, bias=neg40[:])
                pt_b2[kt] = pt

            def softmax_b1(g, j):
                kt = G * g + j
                e1 = sbt.tile([P, R], f32, tag="e1", bufs=4)
                nc.scalar.activation(e1[:], s1bc[0][:], Act.Prelu,
                                     bias=svn_sb[kt][:, 0:1],
                                     alpha=ALPHA)
                z = sbt.tile([P, R], f32, tag="m", bufs=6)
                nc.vector.scalar_tensor_tensor(
                    z[:], adjT_sb[kt // 2][:, kt % 2, :], BIG2, e1[:],
                    op0=Alu.mult, op1=Alu.add)
                pt = sbt.tile([P, R], bf16, tag="pt", bufs=16)
                nc.scalar.activation(pt[:], z[:], Act.Exp, bias=neg40[:])
                pt_b1[kt] = pt

            def emit_att(g):
                for j in range(G):
                    kt = G * g + j
                    nc.tensor.matmul(accT[0][:],
                                     whf_t[0][kt][:, 0:F + 1], pt_b1[kt][:],
                                     start=(kt == 0), stop=(kt == NT - 1))
                    nc.tensor.matmul(accT[1][:],
                                     whf_t[1][kt][:, 0:F + 1], pt_b2[kt][:],
                                     start=(kt == 0), stop=(kt == NT - 1))

            # ---- main loop ----
            for g in range(NG):
                chunk_wh(g)
                if g + 1 < NG:
                    load_af(g + 1)
                cnts = [psC.tile([P, R], f32, tag="cnt", name=f"cnt{g}_{j}")
                        for j in range(G)]
                for t in range(NTP):
                    af = af_tiles.pop((g, t))
                    for j in range(G):
                        nc.tensor.matmul(cnts[j][:],
                                         af[:, :, P * j:P * (j + 1)],
                                         adjT_sb[t][:],
                                         perf_mode=DR,
                                         start=(t == 0), stop=(t == NTP - 1))
                for j in range(G):
                    softmax_b2(g, j, cnts[j])
                for j in range(G):
                    softmax_b1(g, j)
                if g >= 1:
                    emit_att(g - 1)
            emit_att(NG - 1)

            # ---- epilogue: normalize, BN stats + AllReduce, BN+lrelu ----
            hp = []
            for b in range(2):
                srec = sb.tile([F + 1, R], f32, tag=f"srec{b}")
                nc.vector.tensor_copy(srec[F:F + 1, :], accT[b][F:F + 1, :])
                rrec = sb.tile([F + 1, R], f32, tag=f"rrec{b}")
                nc.vector.reciprocal(rrec[F:F + 1, :], srec[F:F + 1, :])
                bc_ps = psC.tile([F, R], f32, tag="cnt", name=f"bc_ps{b}")
                nc.tensor.matmul(bc_ps[:], ones64[F:F + 1, :],
                                 rrec[F:F + 1, :],
                                 start=True, stop=True)
                bc_sb = sb.tile([F, R], f32, tag=f"bcs{b}")
                nc.vector.tensor_copy(bc_sb[:], bc_ps[:])
                hp_b = sb.tile([F, R], f32, tag=f"hp{b}")
                nc.vector.tensor_tensor(hp_b[:], accT[b][0:F, :], bc_sb[:],
                                        op=Alu.mult)
                hp.append(hp_b)

            # stats packed [64, 4]: (sum1, sumsq1, sum2, sumsq2)
            sx = sb.tile([F, 4], f32)
            sq = sb.tile([F, R], bf16)
            for b in range(2):
                nc.vector.tensor_reduce(sx[:, 2 * b:2 * b + 1], hp[b][:],
                                        axis=mybir.AxisListType.X,
                                        op=Alu.add)
                nc.scalar.activation(sq[:], hp[b][:], Act.Square,
                                     accum_out=sx[:, 2 * b + 1:2 * b + 2])
            stats_in = dram.tile([F, 4], f32, name="stats_in")
            nc.sync.dma_start(stats_in[:], sx[:])
            stats_out = dram.tile([F, 4], f32, addr_space="Shared",
                                  name="stats_out")
            nc.gpsimd.collective_compute(
                "AllReduce", Alu.add, replica_groups=RG,
                ins=[stats_in[:].opt()], outs=[stats_out[:].opt()])
            gst = sb.tile([F, 4], f32)
            nc.sync.dma_start(gst[:], stats_out[:])

            gst3 = gst[:].rearrange("f (b s) -> f b s", b=2)
            mean = sb.tile([F, 2], f32)
            nc.scalar.mul(mean[:], gst3[:, :, 0], INV_N)
            ex2 = sb.tile([F, 2], f32)
            nc.scalar.mul(ex2[:], gst3[:, :, 1], INV_N)
            var = sb.tile([F, 2], f32)
            nc.vector.scalar_tensor_tensor(var[:], mean[:], -1.0, mean[:],
                                           op0=Alu.mult, op1=Alu.mult)
            nc.vector.tensor_add(var[:], var[:], ex2[:])
            nc.vector.tensor_scalar_add(var[:], var[:], EPS)
            std = sb.tile([F, 2], f32)
            nc.scalar.activation(std[:], var[:], Act.Sqrt)
            rstd = sb.tile([F, 2], f32)
            nc.vector.reciprocal(rstd[:], std[:])
            gb3 = gb_sb[:].rearrange("f (b s) -> f b s", b=2)
            scale = sb.tile([F, 2], f32)
            nc.vector.tensor_mul(scale[:], gb3[:, :, 0], rstd[:])
            nbias = sb.tile([F, 2], f32)
            nc.vector.scalar_tensor_tensor(nbias[:], mean[:], -1.0, scale[:],
                                           op0=Alu.mult, op1=Alu.mult)
            nc.vector.tensor_add(nbias[:], nbias[:], gb3[:, :, 1])

            # fused BN apply + lrelu; transpose out per branch
            ob = sb.tile([P, RQ, 2 * F], f32)
            for b in range(2):
                finb = sb.tile([F, R], f32, tag=f"fin{b}")
                nc.scalar.activation(finb[:], hp[b][:], Act.Prelu,
                                     bias=nbias[:, b:b + 1],
                                     scale=scale[:, b:b + 1], alpha=ALPHA)
                for q in range(RQ):
                    tp = psC.tile([P, F], f32, tag="cnt", name=f"otp{b}_{q}")
                    nc.tensor.transpose(tp[:], finb[:, P * q:P * (q + 1)],
                                        ident[0:F, 0:F])
                    nc.vector.tensor_copy(ob[:, q, F * b:F * (b + 1)],
                                          tp[:])
            nc.sync.dma_start(
                out_p.rearrange("(q p) f -> p q f", p=P), ob[:])

    nc.compile()
    return nc


def _get_nc():
    if "nc" not in _CACHED:
        _CACHED["nc"] = build_nc()
    return _CACHED["nc"]


def make_in_maps(h, adj, W1, W2, a, gamma, beta):
    h = np.asarray(h, dtype=np.float32)
    adj = np.asarray(adj, dtype=np.float32)
    W1 = np.asarray(W1, np.float32)
    W2 = np.asarray(W2, np.float32)
    a_flat = np.asarray(a, np.float32).reshape(2 * F)
    a1, a2 = a_flat[:F], a_flat[F:]
    # V4 = folded W @ a vectors: s = h @ V4 gives s1/s2 for both branches
    V4 = np.zeros((IN_F, 4), dtype=np.float32)
    V4[:HALF, 0] = W1 @ a1
    V4[HALF:, 1] = W2 @ a1
    V4[:HALF, 2] = W1 @ a2
    V4[HALF:, 3] = W2 @ a2
    # WV [512, 65]: rows 128t:128(t+1) -> branch b = t//2: cols 0:64 the
    # W block for that feature tile, col 64 = folded (W @ a2) weights so
    # the natural-Wh matmul also emits the s2 bias per k
    WV = np.zeros((IN_F, F + 1), dtype=ml_dtypes.bfloat16)
    WV[:HALF, 0:F] = W1
    WV[HALF:, 0:F] = W2
    WV[:, F] = V4[:, 2] + V4[:, 3]
    gamma = np.asarray(gamma, np.float32)
    beta = np.asarray(beta, np.float32)
    gb = np.stack([gamma[:F], beta[:F], gamma[F:], beta[F:]], axis=1)
    ident = np.eye(P, dtype=np.float32)

    fp8 = ml_dtypes.float8_e4m3fn
    adj_f8 = adj.astype(fp8)
    # full adj in DoubleRow layout: adjdr[128T+p, s, k] = adj[256T+128s+p, k]
    adjdr = np.ascontiguousarray(
        adj_f8.reshape(NTP, 2, P, N).transpose(0, 2, 1, 3)
        .reshape(NTP * P, 2, N))
    hTf = np.ascontiguousarray(h.T).astype(ml_dtypes.bfloat16)

    in_maps = []
    for c in range(M_CORES):
        r0 = c * R
        shT = np.ascontiguousarray(adj[r0:r0 + R, :].T).astype(fp8)
        adjT = np.ascontiguousarray(
            shT.reshape(NTP, 2, P, R).transpose(0, 2, 1, 3)
            .reshape(NTP * P, 2, R))
        dts = np.zeros((N, R), dtype=ml_dtypes.bfloat16)
        dts[np.arange(r0, r0 + R), np.arange(R)] = -1.0
        in_maps.append({
            "hT": np.ascontiguousarray(h[r0:r0 + R, :].T),
            "hTf": hTf,
            "adjdr": adjdr,
            "adjT": adjT,
            "dts": dts,
            "WV": WV,
            "V4": V4,
            "gb": gb,
            "ident": ident,
            "ones1r": np.ones((1, P), dtype=np.float32),
        })
    return in_maps


def kernel(h, adj, W1, W2, a, gamma, beta):
    from concourse.bass_utils import run_bass_kernel_spmd

    in_maps = make_in_maps(h, adj, W1, W2, a, gamma, beta)
    nc = _get_nc()
    res = run_bass_kernel_spmd(nc, in_maps, core_ids=list(range(M_CORES)))
    outs = [np.asarray(res.results[c]["out"]) for c in range(M_CORES)]
    return np.concatenate(outs, axis=0)
